# revision 3
# baseline (speedup 1.0000x reference)
"""Trainium2 Bass kernel for nn_EulerCausalAttention_75892072121064.

Sharding: batch*heads across 8 cores (core c -> batch c//4, heads 4*(c%4)..+4).
Each core computes transposed-layout causal attention for its (b, 4-head)
slice plus the out-proj partial, writing outT (D, S). Host sums the 4
per-batch partials and transposes back.

v2 design (vs baseline): ACT was the bottleneck (177us busy of 260us).
- Exp restructured kb-major with 1024-wide query windows: one causal-trimmed
  exp per (head, kb) reading a 2-bank PSUM score tile -> 96 wide ACT
  instructions instead of 160 narrow ones.
- Sin feature path: LUT emulation via magic-number rounding (exact RNE) and
  the ADD_RANGE_WRAP custom DVE op (one op per wrap); 16 Sin calls on
  [64, 2048] tiles. ACT carries no copies at all.
- Causal diag masking via a tiny identity*negtri matmul accumulated into the
  scores PSUM (PE) instead of DVE tensor_tensor multiplies.
- Softmax denominators: V is augmented with a ones column; per head the
  denom row is broadcast (gpsimd), reciprocal'd (2 custom DVE ops), and
  multiplied into the out-proj input; odd heads cross-partition-copied.
- Out-proj per query-half, deferred one head to hide the normalize tail.
"""
import sys

import numpy as np

sys.path.insert(0, "/opt/trn_rl_repo")

from concourse import bacc, mybir  # noqa: E402
import concourse.tile as tile  # noqa: E402
from concourse.bass_utils import run_bass_kernel_spmd  # noqa: E402

B, S, D, H, DH = 2, 2048, 1024, 16, 64
LUT = 4096
TWO_PI = 2.0 * np.pi
SCALE = float(np.sqrt(np.float32(2.0 * DH)))  # sqrt(128) as f32
NCORES = 8
HPC = 4            # heads per core
CW = HPC * DH      # 256 cols per core
QW = 1024          # query window (half of S)
C_LUT = float(np.float32(TWO_PI / LUT))
MAGIC = float(np.float32(12582912.0))  # 1.5*2^23: x+M-M == rne-round(x)
NS = S // 128      # seq tiles

F32 = mybir.dt.float32
F32R = mybir.dt.float32r
I32 = mybir.dt.int32
AF = mybir.ActivationFunctionType
ALU = mybir.AluOpType

BC_FROM_PSUM = False   # gpsimd partition_broadcast reading PSUM directly

_CACHE = {}


def _build_nc():
    nc = bacc.Bacc("TRN2", debug=False, num_devices=NCORES)

    xbT = nc.dram_tensor("xbT", [D, S], F32R, kind="ExternalInput")
    vwT = nc.dram_tensor("vwT", [D, CW], F32R, kind="ExternalInput")
    owT = nc.dram_tensor("owT", [CW, D], F32R, kind="ExternalInput")
    invq = nc.dram_tensor("invq", [128, 2], F32, kind="ExternalInput")
    bq = nc.dram_tensor("bq", [128, 2], F32, kind="ExternalInput")
    invk = nc.dram_tensor("invk", [128, 2], F32, kind="ExternalInput")
    bk = nc.dram_tensor("bk", [128, 2], F32, kind="ExternalInput")
    ngt = nc.dram_tensor("ngt", [128, 128], F32R, kind="ExternalInput")
    idt = nc.dram_tensor("idt", [128, 128], F32R, kind="ExternalInput")
    outT = nc.dram_tensor("outT", [D, S], F32, kind="ExternalOutput")

    inv_scale = float(1.0 / np.float32(SCALE))

    with tile.TileContext(nc) as tc:
        with (
            tc.tile_pool(name="persist", bufs=1) as pp,
            tc.tile_pool(name="qkt", bufs=1) as qkp,
            tc.tile_pool(name="vtiles", bufs=1) as vp,
        ):
            ngt_sb = pp.tile([128, 128], F32R, tag="ngt")
            nc.sync.dma_start(ngt_sb[:], ngt[:])
            idt_sb = pp.tile([128, 128], F32R, tag="idt")
            nc.sync.dma_start(idt_sb[:], idt[:])
            invq_sb = pp.tile([128, 2], F32, tag="invq")
            nc.sync.dma_start(invq_sb[:], invq[:])
            bq_sb = pp.tile([128, 2], F32, tag="bq")
            nc.sync.dma_start(bq_sb[:], bq[:])
            invk_sb = pp.tile([128, 2], F32, tag="invk")
            nc.sync.dma_start(invk_sb[:], invk[:])
            bk_sb = pp.tile([128, 2], F32, tag="bk")
            nc.sync.dma_start(bk_sb[:], bk[:])
            owr = []
            for hp in range(2):
                ow_t = pp.tile([128, D], F32R, tag=f"owr{hp}", name=f"owr{hp}")
                nc.sync.dma_start(ow_t[:], owT[hp * 128:(hp + 1) * 128, :])
                owr.append(ow_t)

            qt = [qkp.tile([128, S], F32R, tag=f"qt{h}", name=f"qt{h}")
                  for h in range(HPC)]
            kt = [qkp.tile([128, S], F32R, tag=f"kt{h}", name=f"kt{h}")
                  for h in range(HPC)]
            vt = [vp.tile([128, HPC * 65], F32R, tag=f"v{s}", name=f"v{s}")
                  for s in range(NS)]

            # ---- phase 1: features (DVE chain + Sin) and V projection ----
            with (
                tc.tile_pool(name="xtp", bufs=1) as xtp,
                tc.tile_pool(name="chain", bufs=1) as chp,
                tc.tile_pool(name="vwp", bufs=1) as vwp,
                tc.tile_pool(name="v_ps", bufs=2, space="PSUM") as vps,
            ):
                xT = []
                for od in range(8):
                    x_t = xtp.tile([128, S], F32R, tag=f"xT{od}",
                                   name=f"xT{od}")
                    nc.sync.dma_start(x_t[:], xbT[od * 128:(od + 1) * 128, :])
                    xT.append(x_t)

                def chain(t, inv_sb, b_sb, dsts):
                    # theta*s with per-feature scale/bias
                    ts2 = chp.tile([128, S], F32, tag="chA", name="ts2",
                                   bufs=2)
                    nc.vector.tensor_scalar(
                        ts2[:], xT[t][:], inv_sb[:, t:t + 1], b_sb[:, t:t + 1],
                        ALU.mult, ALU.add,
                    )
                    # k = round(theta*s), exact RNE via magic add/sub
                    kf = chp.tile([128, S], F32, tag="chB", name="kf", bufs=1)
                    nc.vector.tensor_scalar(kf[:], ts2[:], MAGIC, MAGIC,
                                            ALU.add, ALU.subtract)
                    # sin arg: wrap k into [-2048, 2048]
                    kwS = chp.tile([128, S], F32, tag="chA", name="kwS",
                                   bufs=2)
                    nc.vector.add_range_wrap(kwS[:], kf[:], 0.0, 2048.0,
                                             4096.0)
                    # cos arg: shift by 1024 (pi/2) then wrap
                    kwC = chp.tile([128, S], F32, tag="chC", name="kwC",
                                   bufs=1)
                    nc.vector.add_range_wrap(kwC[:], kf[:], 1024.0, 2048.0,
                                             4096.0)
                    for hh in range(2):
                        dtile = dsts[2 * t + hh]
                        rows = slice(hh * 64, hh * 64 + 64)
                        nc.scalar.activation(dtile[0:64, :], kwC[rows, :],
                                             AF.Sin, scale=C_LUT)
                        nc.scalar.activation(dtile[64:128, :], kwS[rows, :],
                                             AF.Sin, scale=C_LUT)

                chain(0, invq_sb, bq_sb, qt)
                chain(0, invk_sb, bk_sb, kt)

                # V = x @ vwT, augmented with a ones column per head
                vwr = []
                for od in range(8):
                    vw_t = vwp.tile([128, CW], F32R, tag=f"vwr{od}",
                                    name=f"vwr{od}")
                    nc.sync.dma_start(vw_t[:],
                                      vwT[od * 128:(od + 1) * 128, :])
                    vwr.append(vw_t)
                for si in range(NS):
                    vpsum = vps.tile([128, CW], F32, tag="vpsum",
                                     name="vpsum")
                    for od in range(8):
                        nc.tensor.matmul(
                            vpsum[:],
                            xT[od][:, si * 128:(si + 1) * 128],
                            vwr[od][:],
                            start=(od == 0), stop=(od == 7),
                        )
                    dst = vt[si][:].rearrange(
                        "p (h w) -> p h w", w=65)[:, :, 0:64]
                    src = vpsum[:].rearrange("p (h w) -> p h w", w=64)
                    nc.vector.tensor_copy(dst, src)
                    onescol = vt[si][:].rearrange(
                        "p (h w) -> p h w", w=65)[:, :, 64:65]
                    nc.gpsimd.memset(onescol.bitcast(F32), 1.0)

                chain(1, invq_sb, bq_sb, qt)
                chain(1, invk_sb, bk_sb, kt)

            # ---- phase 2: attention + out projection ----
            with (
                tc.tile_pool(name="atp", bufs=1) as ap,
                tc.tile_pool(name="osb", bufs=1) as op,
                tc.tile_pool(name="sc_ps", bufs=2, space="PSUM") as scp,
                tc.tile_pool(name="o_ps", bufs=2, space="PSUM") as opp,
            ):
                pairs = {}  # (qh, hp) -> tile

                def outproj(qh):
                    qlo = QW * qh
                    for od in range(8):
                        pr = scp.tile([128, QW], F32, tag="sc", name="pr")
                        for c2 in range(2):
                            cs = slice(c2 * 512, c2 * 512 + 512)
                            for hp in range(2):
                                nc.tensor.matmul(
                                    pr[:, cs],
                                    owr[hp][:, od * 128:(od + 1) * 128],
                                    pairs[(qh, hp)][:, cs],
                                    start=(hp == 0), stop=(hp == 1),
                                )
                        prsb = op.tile([128, QW], F32, tag="prsb",
                                       name="prsb", bufs=2)
                        nc.vector.tensor_copy(prsb[:], pr[:])
                        nc.sync.dma_start(
                            outT[od * 128:(od + 1) * 128, qlo:qlo + QW],
                            prsb[:],
                        )

                for qh in range(2):
                    qlo = QW * qh
                    kbmax = 8 * qh + 8
                    for hp in range(2):
                        pairs[(qh, hp)] = op.tile(
                            [128, QW], F32R, tag=f"pairs{hp}",
                            name=f"pairs{qh}{hp}", bufs=2)
                    for h in range(HPC):
                        o_ps = opp.tile([65, QW], F32, tag="o", name="o_ps")
                        for kb in range(kbmax):
                            vcol = max(0, 128 * kb - qlo)
                            j0 = vcol // 512
                            dc = 128 * kb - qlo  # diag col if in window
                            sc = scp.tile([128, QW], F32, tag="sc", name="sc")
                            for j in range(j0, 2):
                                jdiag = (kb // 8 == qh) and (dc // 512 == j)
                                nc.tensor.matmul(
                                    sc[:, j * 512:(j + 1) * 512],
                                    kt[h][:, kb * 128:(kb + 1) * 128],
                                    qt[h][:, qlo + j * 512:qlo + j * 512 + 512],
                                    start=True, stop=not jdiag,
                                )
                                if jdiag:
                                    nc.tensor.matmul(
                                        sc[:, dc:dc + 128], idt_sb[:],
                                        ngt_sb[:], start=False, stop=True,
                                    )
                            at = ap.tile([128, QW], F32R, tag="at", name="at",
                                         bufs=3)
                            nc.scalar.activation(
                                at[:, vcol:QW], sc[:, vcol:QW], AF.Exp,
                                scale=inv_scale,
                            )
                            if vcol % 512:
                                nc.gpsimd.memset(
                                    at[:, 512 * j0:vcol].bitcast(F32), 0.0)
                            for j in range(j0, 2):
                                nc.tensor.matmul(
                                    o_ps[:, j * 512:(j + 1) * 512],
                                    vt[kb][:, h * 65:(h + 1) * 65],
                                    at[:, j * 512:(j + 1) * 512],
                                    start=(kb == 0),
                                    stop=(kb == 8 * qh + 4 * j + 3),
                                )
                        # normalize: denom row 64 -> bc -> recip -> mult
                        bc = op.tile([64, QW], F32, tag="bc", name="bc",
                                     bufs=2)
                        if BC_FROM_PSUM:
                            nc.gpsimd.partition_broadcast(bc[:],
                                                          o_ps[64:65, :])
                        else:
                            srow = op.tile([1, QW], F32, tag="srow",
                                           name="srow", bufs=2)
                            nc.vector.tensor_copy(srow[:], o_ps[64:65, :])
                            nc.gpsimd.partition_broadcast(bc[:], srow[:])
                        rec = op.tile([64, QW], F32, tag="rec", name="rec",
                                      bufs=2)
                        scr = op.tile([64, QW], F32, tag="scr", name="scr",
                                      bufs=2)
                        nc.vector.reciprocal_approx_accurate(rec[:], bc[:],
                                                             scr[:])
                        dstp = pairs[(qh, h // 2)]
                        if h % 2 == 0:
                            nc.vector.tensor_tensor(
                                dstp[0:64, :], o_ps[0:64, :], rec[:],
                                ALU.mult)
                        else:
                            tmp = op.tile([64, QW], F32R, tag="tmp",
                                          name="tmp", bufs=2)
                            nc.vector.tensor_tensor(
                                tmp[:], o_ps[0:64, :], rec[:], ALU.mult)
                            nc.vector.tensor_copy(dstp[64:128, :], tmp[:])
                        if qh == 1 and h == 0:
                            outproj(0)  # deferred: pairs(0,*) long ready
                outproj(1)

    nc.compile()
    return nc


def _prep_inputs(x, w_q, b_q, w_k, b_k, v_w, out_w):
    """Build the 8 per-core input maps (host-side sharding)."""
    s_lut = np.float64(LUT) / TWO_PI
    in_maps = []
    ngt = np.where(np.arange(128)[None, :] < np.arange(128)[:, None],
                   np.float32(-1e5), np.float32(0.0)).astype(np.float32)
    idt = np.eye(128, dtype=np.float32)

    wq = w_q.reshape(D)
    bqv = b_q.reshape(D)
    wk = w_k.reshape(D)
    bkv = b_k.reshape(D)

    for c in range(NCORES):
        b = c // 4
        h0 = (c % 4) * HPC
        colbase = h0 * DH
        cols = np.arange(colbase, colbase + CW)
        rest = np.concatenate([np.arange(0, colbase),
                               np.arange(colbase + CW, D)])
        perm = np.concatenate([cols, rest])

        xbT = np.ascontiguousarray(x[b][:, perm].T, dtype=np.float32)
        vwT = np.ascontiguousarray(v_w[cols][:, perm].T, dtype=np.float32)
        owT = np.ascontiguousarray(out_w[:, cols].T, dtype=np.float32)

        def featparams(w, bias):
            inv = s_lut / (1.0 + np.abs(w[cols].astype(np.float64)))
            bb = bias[cols].astype(np.float64) * s_lut
            return (inv.reshape(2, 128).T.astype(np.float32).copy(),
                    bb.reshape(2, 128).T.astype(np.float32).copy())

        iq, bq_ = featparams(wq, bqv)
        ik, bk_ = featparams(wk, bkv)

        in_maps.append(dict(
            xbT=xbT, vwT=vwT, owT=owT,
            invq=iq, bq=bq_, invk=ik, bk=bk_,
            ngt=ngt, idt=idt,
        ))
    return in_maps


def kernel(x, w_q, b_q, w_k, b_k, v_w, out_w, _trace=False):
    x = np.asarray(x, dtype=np.float32)
    w_q = np.asarray(w_q, dtype=np.float32)
    b_q = np.asarray(b_q, dtype=np.float32)
    w_k = np.asarray(w_k, dtype=np.float32)
    b_k = np.asarray(b_k, dtype=np.float32)
    v_w = np.asarray(v_w, dtype=np.float32)
    out_w = np.asarray(out_w, dtype=np.float32)

    if "nc" not in _CACHE:
        _CACHE["nc"] = _build_nc()
    nc = _CACHE["nc"]

    in_maps = _prep_inputs(x, w_q, b_q, w_k, b_k, v_w, out_w)
    res = run_bass_kernel_spmd(
        nc, in_maps, core_ids=list(range(NCORES)), trace=_trace
    )
    out = np.zeros((B, S, D), dtype=np.float32)
    for c in range(NCORES):
        out[c // 4] += res.results[c]["outT"].T
    if _trace:
        kernel._last_result = res
    return out


# revision 5
# speedup vs baseline: 1.2328x; 1.2328x over previous
"""Trainium2 Bass kernel for nn_EulerCausalAttention_75892072121064.

Sharding: batch*heads across 8 cores (core c -> batch c//4, heads 4*(c%4)..+4).
Each core computes transposed-layout causal attention for its (b, 4-head)
slice plus the out-proj partial, writing outT (D, S). Host sums the 4
per-batch partials and transposes back.

v3 design:
- Exp kb-major with 1024-wide query windows: one causal-trimmed exp per
  (head, kb) reading a 2-bank PSUM score tile (96 wide ACT instructions).
- kb loop software-pipelined: AV matmuls for kb-1 are emitted after the
  score matmuls for kb so the in-order PE queue never stalls on the exp.
- Score/AV matmul free dims causally trimmed to the valid query range.
- Sin feature path: LUT emulation via magic-number rounding (exact RNE) +
  ADD_RANGE_WRAP custom DVE op; 16 Sin calls on [64, 2048]; ACT carries no
  copies. All four chains emitted back-to-back so the DVE queue feeds the
  Sin stream before the (DMA-gated) V copies.
- V = x @ vwT with a ones column per head (softmax denominator comes out of
  the attn@V accumulation); V PSUM drains on gpsimd to keep DVE free.
- Normalize per head: denom row -> broadcast -> reciprocal (custom DVE) ->
  multiply; odd heads cross-partition-copied into the head-pair tile.
- Out-proj per query-half, deferred one head to hide the normalize tail.
"""
import sys

import numpy as np

sys.path.insert(0, "/opt/trn_rl_repo")

from concourse import bacc, mybir  # noqa: E402
import concourse.tile as tile  # noqa: E402
from concourse.bass_utils import run_bass_kernel_spmd  # noqa: E402

B, S, D, H, DH = 2, 2048, 1024, 16, 64
LUT = 4096
TWO_PI = 2.0 * np.pi
SCALE = float(np.sqrt(np.float32(2.0 * DH)))  # sqrt(128) as f32
NCORES = 8
HPC = 4            # heads per core
CW = HPC * DH      # 256 cols per core
QW = 1024          # query window (half of S)
C_LUT = float(np.float32(TWO_PI / LUT))
MAGIC = float(np.float32(12582912.0))  # 1.5*2^23: x+M-M == rne-round(x)
NS = S // 128      # seq tiles

F32 = mybir.dt.float32
F32R = mybir.dt.float32r
I32 = mybir.dt.int32
AF = mybir.ActivationFunctionType
ALU = mybir.AluOpType

_CACHE = {}


def _build_nc():
    nc = bacc.Bacc("TRN2", debug=False, num_devices=NCORES)

    xbT = nc.dram_tensor("xbT", [D, S], F32R, kind="ExternalInput")
    vwT = nc.dram_tensor("vwT", [D, CW], F32R, kind="ExternalInput")
    owT = nc.dram_tensor("owT", [CW, D], F32R, kind="ExternalInput")
    invq = nc.dram_tensor("invq", [128, 2], F32, kind="ExternalInput")
    bq = nc.dram_tensor("bq", [128, 2], F32, kind="ExternalInput")
    invk = nc.dram_tensor("invk", [128, 2], F32, kind="ExternalInput")
    bk = nc.dram_tensor("bk", [128, 2], F32, kind="ExternalInput")
    tri = nc.dram_tensor("tri", [128, 128], F32R, kind="ExternalInput")
    outT = nc.dram_tensor("outT", [D, S], F32, kind="ExternalOutput")

    inv_scale = float(1.0 / np.float32(SCALE))

    with tile.TileContext(nc) as tc:
        with (
            tc.tile_pool(name="persist", bufs=1) as pp,
            tc.tile_pool(name="qkt", bufs=1) as qkp,
            tc.tile_pool(name="vtiles", bufs=1) as vp,
        ):
            invq_sb = pp.tile([128, 2], F32, tag="invq")
            nc.sync.dma_start(invq_sb[:], invq[:])
            bq_sb = pp.tile([128, 2], F32, tag="bq")
            nc.sync.dma_start(bq_sb[:], bq[:])
            invk_sb = pp.tile([128, 2], F32, tag="invk")
            nc.sync.dma_start(invk_sb[:], invk[:])
            bk_sb = pp.tile([128, 2], F32, tag="bk")
            nc.sync.dma_start(bk_sb[:], bk[:])
            tri_sb = pp.tile([128, 128], F32R, tag="tri")
            nc.sync.dma_start(tri_sb[:], tri[:])

            qt = [qkp.tile([128, S], F32R, tag=f"qt{h}", name=f"qt{h}")
                  for h in range(HPC)]
            kt = [qkp.tile([128, S], F32R, tag=f"kt{h}", name=f"kt{h}")
                  for h in range(HPC)]
            vt = [vp.tile([128, HPC * 65], F32R, tag=f"v{s}", name=f"v{s}")
                  for s in range(NS)]

            # ---- phase 1: features (DVE chain + Sin) and V projection ----
            with (
                tc.tile_pool(name="xtp", bufs=1) as xtp,
                tc.tile_pool(name="chain", bufs=1) as chp,
                tc.tile_pool(name="vwp", bufs=1) as vwp,
                tc.tile_pool(name="v_ps", bufs=2, space="PSUM") as vps,
            ):
                xT = []
                for od in range(8):
                    x_t = xtp.tile([128, S], F32R, tag=f"xT{od}",
                                   name=f"xT{od}")
                    nc.sync.dma_start(x_t[:], xbT[od * 128:(od + 1) * 128, :])
                    xT.append(x_t)

                def chain(t, inv_sb, b_sb, dsts):
                    # theta*s with per-feature scale/bias
                    ts2 = chp.tile([128, S], F32, tag="chA", name="ts2",
                                   bufs=2)
                    nc.vector.tensor_scalar(
                        ts2[:], xT[t][:], inv_sb[:, t:t + 1], b_sb[:, t:t + 1],
                        ALU.mult, ALU.add,
                    )
                    # k = round(theta*s), exact RNE via magic add/sub
                    kf = chp.tile([128, S], F32, tag="chB", name="kf", bufs=1)
                    nc.vector.tensor_scalar(kf[:], ts2[:], MAGIC, MAGIC,
                                            ALU.add, ALU.subtract)
                    # sin arg: wrap k into [-2048, 2048]
                    kwS = chp.tile([128, S], F32, tag="chA", name="kwS",
                                   bufs=2)
                    nc.vector.add_range_wrap(kwS[:], kf[:], 0.0, 2048.0,
                                             4096.0)
                    # cos arg: shift by 1024 (pi/2) then wrap
                    kwC = chp.tile([128, S], F32, tag="chC", name="kwC",
                                   bufs=1)
                    nc.vector.add_range_wrap(kwC[:], kf[:], 1024.0, 2048.0,
                                             4096.0)
                    for hh in range(2):
                        dtile = dsts[2 * t + hh]
                        rows = slice(hh * 64, hh * 64 + 64)
                        nc.scalar.activation(dtile[0:64, :], kwC[rows, :],
                                             AF.Sin, scale=C_LUT)
                        nc.scalar.activation(dtile[64:128, :], kwS[rows, :],
                                             AF.Sin, scale=C_LUT)

                chain(0, invq_sb, bq_sb, qt)
                chain(0, invk_sb, bk_sb, kt)
                chain(1, invq_sb, bq_sb, qt)
                chain(1, invk_sb, bk_sb, kt)

                # V = x @ vwT, augmented with a ones column per head
                vwr = []
                for od in range(8):
                    vw_t = vwp.tile([128, CW], F32R, tag=f"vwr{od}",
                                    name=f"vwr{od}")
                    nc.sync.dma_start(vw_t[:],
                                      vwT[od * 128:(od + 1) * 128, :])
                    vwr.append(vw_t)
                for si in range(NS):
                    vpsum = vps.tile([128, CW], F32, tag="vpsum",
                                     name="vpsum")
                    for od in range(8):
                        nc.tensor.matmul(
                            vpsum[:],
                            xT[od][:, si * 128:(si + 1) * 128],
                            vwr[od][:],
                            start=(od == 0), stop=(od == 7),
                        )
                    dst = vt[si][:].rearrange(
                        "p (h w) -> p h w", w=65)[:, :, 0:64]
                    src = vpsum[:].rearrange("p (h w) -> p h w", w=64)
                    nc.vector.tensor_copy(dst, src)
                    onescol = vt[si][:].rearrange(
                        "p (h w) -> p h w", w=65)[:, :, 64:65]
                    nc.gpsimd.memset(onescol.bitcast(F32), 1.0)

            # out-proj weights, needed late
            owr = []
            for hp in range(2):
                ow_t = pp.tile([128, D], F32R, tag=f"owr{hp}", name=f"owr{hp}")
                nc.sync.dma_start(ow_t[:], owT[hp * 128:(hp + 1) * 128, :])
                owr.append(ow_t)

            # ---- phase 2: attention + out projection ----
            with (
                tc.tile_pool(name="atp", bufs=1) as ap,
                tc.tile_pool(name="osb", bufs=1) as op,
                tc.tile_pool(name="sc_ps", bufs=2, space="PSUM") as scp,
                tc.tile_pool(name="o_ps", bufs=2, space="PSUM") as opp,
            ):
                pairs = {}  # (qh, hp) -> tile

                def outproj(qh):
                    qlo = QW * qh
                    for od in range(8):
                        pr = scp.tile([128, QW], F32, tag="sc", name="pr")
                        for c2 in range(2):
                            cs = slice(c2 * 512, c2 * 512 + 512)
                            for hp in range(2):
                                nc.tensor.matmul(
                                    pr[:, cs],
                                    owr[hp][:, od * 128:(od + 1) * 128],
                                    pairs[(qh, hp)][:, cs],
                                    start=(hp == 0), stop=(hp == 1),
                                )
                        prsb = op.tile([128, QW], F32, tag="prsb",
                                       name="prsb", bufs=2)
                        nc.vector.tensor_copy(prsb[:], pr[:])
                        nc.sync.dma_start(
                            outT[od * 128:(od + 1) * 128, qlo:qlo + QW],
                            prsb[:],
                        )

                for qh in range(2):
                    qlo = QW * qh
                    kbmax = 8 * qh + 8
                    for hp in range(2):
                        pairs[(qh, hp)] = op.tile(
                            [128, QW], F32R, tag=f"pairs{hp}",
                            name=f"pairs{qh}{hp}", bufs=2)
                    for h in range(HPC):
                        o_ps = opp.tile([65, QW], F32, tag="o", name="o_ps")
                        pend = None  # (at, vcol, j0, kb)

                        def flush_av():
                            p_at, p_vcol, p_j0, p_kb = pend
                            for j in range(p_j0, 2):
                                n0 = max(p_vcol, j * 512)
                                n1 = (j + 1) * 512
                                nc.tensor.matmul(
                                    o_ps[:, n0:n1],
                                    vt[p_kb][:, h * 65:(h + 1) * 65],
                                    p_at[:, n0:n1],
                                    start=(p_kb == 0),
                                    stop=(p_kb == 8 * qh + 4 * j + 3),
                                )

                        for kb in range(kbmax):
                            vcol = max(0, 128 * kb - qlo)
                            j0 = vcol // 512
                            dc = 128 * kb - qlo  # diag col if in window
                            sc = scp.tile([128, QW], F32, tag="sc", name="sc")
                            for j in range(j0, 2):
                                n0 = max(vcol, j * 512)
                                n1 = (j + 1) * 512
                                nc.tensor.matmul(
                                    sc[:, n0:n1],
                                    kt[h][:, kb * 128:(kb + 1) * 128],
                                    qt[h][:, qlo + n0:qlo + n1],
                                    start=True, stop=True,
                                )
                            if pend is not None:
                                flush_av()
                            at = ap.tile([128, QW], F32R, tag="at", name="at",
                                         bufs=3)
                            nc.scalar.activation(
                                at[:, vcol:QW], sc[:, vcol:QW], AF.Exp,
                                scale=inv_scale,
                            )
                            if kb // 8 == qh:
                                # mask the diagonal block (keep q >= k)
                                nc.vector.tensor_tensor(
                                    at[:, dc:dc + 128], at[:, dc:dc + 128],
                                    tri_sb[:], ALU.mult,
                                )
                            pend = (at, vcol, j0, kb)
                        flush_av()
                        # normalize: denom row 64 -> bc -> recip -> mult
                        srow = op.tile([1, QW], F32, tag="srow",
                                       name="srow", bufs=2)
                        nc.vector.tensor_copy(srow[:], o_ps[64:65, :])
                        bc = op.tile([64, QW], F32, tag="bc", name="bc",
                                     bufs=2)
                        nc.gpsimd.partition_broadcast(bc[:], srow[:])
                        rec = op.tile([64, QW], F32, tag="rec", name="rec",
                                      bufs=2)
                        scr = op.tile([64, QW], F32, tag="scr", name="scr",
                                      bufs=2)
                        nc.vector.reciprocal_approx_accurate(rec[:], bc[:],
                                                             scr[:])
                        dstp = pairs[(qh, h // 2)]
                        if h % 2 == 0:
                            nc.vector.tensor_tensor(
                                dstp[0:64, :], o_ps[0:64, :], rec[:],
                                ALU.mult)
                        else:
                            tmp = op.tile([64, QW], F32R, tag="tmp",
                                          name="tmp", bufs=2)
                            nc.vector.tensor_tensor(
                                tmp[:], o_ps[0:64, :], rec[:], ALU.mult)
                            nc.vector.tensor_copy(dstp[64:128, :], tmp[:])
                        if qh == 1 and h == 0:
                            outproj(0)  # deferred: pairs(0,*) long ready
                outproj(1)

    nc.compile()
    return nc


def _prep_inputs(x, w_q, b_q, w_k, b_k, v_w, out_w):
    """Build the 8 per-core input maps (host-side sharding)."""
    s_lut = np.float64(LUT) / TWO_PI
    in_maps = []
    tri = np.triu(np.ones((128, 128), dtype=np.float32))  # keep q >= k

    wq = w_q.reshape(D)
    bqv = b_q.reshape(D)
    wk = w_k.reshape(D)
    bkv = b_k.reshape(D)

    for c in range(NCORES):
        b = c // 4
        h0 = (c % 4) * HPC
        colbase = h0 * DH
        cols = np.arange(colbase, colbase + CW)
        rest = np.concatenate([np.arange(0, colbase),
                               np.arange(colbase + CW, D)])
        perm = np.concatenate([cols, rest])

        xbT = np.ascontiguousarray(x[b][:, perm].T, dtype=np.float32)
        vwT = np.ascontiguousarray(v_w[cols][:, perm].T, dtype=np.float32)
        owT = np.ascontiguousarray(out_w[:, cols].T, dtype=np.float32)

        def featparams(w, bias):
            inv = s_lut / (1.0 + np.abs(w[cols].astype(np.float64)))
            bb = bias[cols].astype(np.float64) * s_lut
            return (inv.reshape(2, 128).T.astype(np.float32).copy(),
                    bb.reshape(2, 128).T.astype(np.float32).copy())

        iq, bq_ = featparams(wq, bqv)
        ik, bk_ = featparams(wk, bkv)

        in_maps.append(dict(
            xbT=xbT, vwT=vwT, owT=owT,
            invq=iq, bq=bq_, invk=ik, bk=bk_,
            tri=tri,
        ))
    return in_maps


def kernel(x, w_q, b_q, w_k, b_k, v_w, out_w, _trace=False):
    x = np.asarray(x, dtype=np.float32)
    w_q = np.asarray(w_q, dtype=np.float32)
    b_q = np.asarray(b_q, dtype=np.float32)
    w_k = np.asarray(w_k, dtype=np.float32)
    b_k = np.asarray(b_k, dtype=np.float32)
    v_w = np.asarray(v_w, dtype=np.float32)
    out_w = np.asarray(out_w, dtype=np.float32)

    if "nc" not in _CACHE:
        _CACHE["nc"] = _build_nc()
    nc = _CACHE["nc"]

    in_maps = _prep_inputs(x, w_q, b_q, w_k, b_k, v_w, out_w)
    res = run_bass_kernel_spmd(
        nc, in_maps, core_ids=list(range(NCORES)), trace=_trace
    )
    out = np.zeros((B, S, D), dtype=np.float32)
    for c in range(NCORES):
        out[c // 4] += res.results[c]["outT"].T
    if _trace:
        kernel._last_result = res
    return out
